# revision 3
# baseline (speedup 1.0000x reference)
"""Batched Viterbi decode (CRF) on 8 Trainium2 NeuronCores — v3.

Same contract as the baseline (device computes bit-exact fp32 t1 history,
host backtracks).  Changes vs baseline:
  - adds: 2 ACT bias-adds + 1 DVE pair-add (free-broadcast AP) per 4-seq
    group, cutting the serial-ACT depth in each group's dependency cycle
    from 3 to 2 and total DVE instruction count
  - optional fp32r-mode PE transposes (1.5 cycles/row vs 2.0)
  - t1 history streamed to DRAM in chunks during the loop instead of one
    epilogue burst

Env knobs:
  V3_PAIR=1   use DVE pair-adds (else per-seq adds, ADD_ENG pattern)
  V3_F32R=1   bitcast transposes to float32r
  V3_ADD      fallback per-seq engine pattern when V3_PAIR=0
"""

import os
from contextlib import ExitStack

import numpy as np

S = 128
T = 2048
NS = 8
N_CORES = 8
B = NS * N_CORES

USE_PAIR = os.environ.get("V3_PAIR", "1") == "1"
USE_F32R = os.environ.get("V3_F32R", "0") == "1"
ADD_ENG = os.environ.get("V3_ADD", "aavaavaa")

_CACHE = {}


def _build_forward():
    import concourse.bacc as bacc
    import concourse.mybir as mybir
    import concourse.tile as tile

    F32 = mybir.dt.float32
    F32R = mybir.dt.float32r
    nc = bacc.Bacc("TRN2", num_devices=N_CORES)
    trans_in = nc.dram_tensor("transitions", [NS, S + 1, S], F32, kind="ExternalInput")
    em_in = nc.dram_tensor("emissions", [NS, T, S], F32, kind="ExternalInput")
    ident_in = nc.dram_tensor("identity", [S, S], F32, kind="ExternalInput")
    t1_out = nc.dram_tensor("t1hist", [S, T * NS], F32, kind="ExternalOutput")

    def tp(out_ap, in_ap, id_ap):
        if USE_F32R:
            nc.tensor.transpose(out_ap.bitcast(F32R), in_ap.bitcast(F32R),
                                id_ap.bitcast(F32R))
        else:
            nc.tensor.transpose(out_ap, in_ap, id_ap)

    with ExitStack() as ctx:
        trans_sb = ctx.enter_context(nc.sbuf_tensor([S, NS * S], F32))
        em_cols = ctx.enter_context(nc.sbuf_tensor([S, T * NS], F32))
        t1hist = ctx.enter_context(nc.sbuf_tensor([S, T * NS], F32))
        ident = ctx.enter_context(nc.sbuf_tensor([S, S], F32))
        start_sb = ctx.enter_context(nc.sbuf_tensor([S, NS], F32))
        em0_sb = ctx.enter_context(nc.sbuf_tensor([S, NS], F32))
        psum_pp = [
            [
                ctx.enter_context(nc.psum_tensor(f"pspp{k}g{g}", [S, NS * S // 2], F32))
                for g in range(2)
            ]
            for k in range(3)
        ]

        with tile.TileContext(nc) as tc, ExitStack() as pctx:
            sc_pool = pctx.enter_context(tc.tile_pool(name="scores", bufs=3))
            tmp_pool = pctx.enter_context(tc.tile_pool(name="tmp", bufs=3))
            stage_pool = pctx.enter_context(tc.tile_pool(name="stage", bufs=4))
            pst_pool = pctx.enter_context(tc.tile_pool(name="pst", bufs=2, space="PSUM"))

            # ---- prologue: transitions, identity, t1_0 ----
            for s in range(NS):
                nc.sync.dma_start(trans_sb[:, s * S:(s + 1) * S], trans_in[s, 0:S, :])
            nc.sync.dma_start(ident[:], ident_in[:])
            for s in range(NS):
                nc.sync.dma_start(
                    start_sb[:, s:s + 1], trans_in[s, S:S + 1, :].rearrange("o p -> p o")
                )
                nc.sync.dma_start(
                    em0_sb[:, s:s + 1], em_in[s, 0:1, :].rearrange("o p -> p o")
                )
            nc.vector.tensor_add(t1hist[:, 0:NS], start_sb[:], em0_sb[:])

            # ---- prologue: transpose emissions into em_cols[i, t*NS+s] ----
            for s in range(NS):
                for c in range(T // S):
                    stage = stage_pool.tile([S, S], F32, tag="emstage")
                    nc.sync.dma_start(stage[:], em_in[s, c * S:(c + 1) * S, :])
                    pst = pst_pool.tile([S, S], F32, tag="empsum")
                    nc.tensor.transpose(pst[:], stage[:], ident[:])
                    dst = em_cols[:, c * S * NS + s: (c + 1) * S * NS: NS]
                    nc.scalar.copy(dst, pst[:])

            # ---- main DP loop ----
            def step(t):
                base = (t - 1) * NS
                sc_tiles = {}
                for g in range(2):
                    s0 = g * 4
                    if USE_PAIR:
                        # 2 ACT singles + 1 DVE pair-add per group
                        for s in (s0, s0 + 1):
                            sc = sc_pool.tile([S, S], F32, tag=f"sc{s}")
                            nc.scalar.activation(
                                sc[:], trans_sb[:, s * S:(s + 1) * S],
                                mybir.ActivationFunctionType.Identity,
                                bias=t1hist[:, base + s:base + s + 1], scale=1.0,
                            )
                            sc_tiles[s] = sc
                        scp = sc_pool.tile([S, 2 * S], F32, tag=f"scp{g}")
                        tr2 = trans_sb[:, (s0 + 2) * S:(s0 + 4) * S].rearrange(
                            "p (s i) -> p s i", i=S)
                        t1p = t1hist[:, base + s0 + 2:base + s0 + 4, None].to_broadcast(
                            [S, 2, S])
                        nc.vector.tensor_add(
                            scp[:].rearrange("p (s i) -> p s i", i=S), tr2, t1p)
                        sc_tiles[s0 + 2] = None
                        sc_tiles[s0 + 3] = None
                        sc_tiles[(g, "pair")] = scp
                    else:
                        for s in range(s0, s0 + 4):
                            sc = sc_pool.tile([S, S], F32, tag=f"sc{s}")
                            t1col = t1hist[:, base + s:base + s + 1]
                            src = trans_sb[:, s * S:(s + 1) * S]
                            if ADD_ENG[s] == "a":
                                nc.scalar.activation(
                                    sc[:], src, mybir.ActivationFunctionType.Identity,
                                    bias=t1col, scale=1.0,
                                )
                            else:
                                nc.vector.tensor_scalar_add(sc[:], src, t1col)
                            sc_tiles[s] = sc

                tmp = tmp_pool.tile([S, NS], F32, tag="tmp")
                for g in range(2):
                    pst = psum_pp[t % 3][g]
                    for sl in range(4):
                        s = g * 4 + sl
                        if USE_PAIR and sl >= 2:
                            scp = sc_tiles[(g, "pair")]
                            src = scp[:, (sl - 2) * S:(sl - 1) * S]
                        else:
                            src = sc_tiles[s][:]
                        tp(pst[:, sl * S:(sl + 1) * S], src, ident[:])
                    pg = pst[:].rearrange("p (s i) -> p s i", i=S)
                    nc.vector.tensor_reduce(
                        tmp[:, g * 4:(g + 1) * 4], pg,
                        axis=mybir.AxisListType.X, op=mybir.AluOpType.max)
                    nc.vector.tensor_add(
                        t1hist[:, t * NS + g * 4:t * NS + (g + 1) * 4],
                        tmp[:, g * 4:(g + 1) * 4],
                        em_cols[:, t * NS + g * 4:t * NS + (g + 1) * 4])

            CHUNK = 256
            for t in range(1, T):
                step(t)
                if t % CHUNK == 0:
                    lo = (t - CHUNK) * NS
                    nc.sync.dma_start(
                        t1_out[:, lo:t * NS], t1hist[:, lo:t * NS])

            lo = (T // CHUNK * CHUNK - CHUNK) * NS
            nc.sync.dma_start(t1_out[:, lo:], t1hist[:, lo:])

    nc.finalize()
    return nc


def _get_nc():
    if "nc" not in _CACHE:
        _CACHE["nc"] = _build_forward()
    return _CACHE["nc"]


def kernel(transitions, emissions, lengths):
    from concourse.bass_utils import run_bass_kernel_spmd

    transitions = np.ascontiguousarray(transitions, dtype=np.float32)
    emissions = np.ascontiguousarray(emissions, dtype=np.float32)
    lengths = np.asarray(lengths, dtype=np.int32)
    assert transitions.shape == (B, S + 1, S)
    assert emissions.shape == (B, T, S)

    nc = _get_nc()
    eye = np.eye(S, dtype=np.float32)
    in_maps = [
        {
            "transitions": transitions[c * NS:(c + 1) * NS],
            "emissions": emissions[c * NS:(c + 1) * NS],
            "identity": eye,
        }
        for c in range(N_CORES)
    ]
    res = run_bass_kernel_spmd(
        nc, in_maps, core_ids=list(range(N_CORES)),
        trace=bool(os.environ.get("VIT_TRACE")),
    )
    if os.environ.get("VIT_TRACE"):
        _CACHE["last_exec_time_ns"] = res.exec_time_ns
        _CACHE["last_res"] = res

    t1 = np.empty((B, T, S), dtype=np.float32)
    for c in range(N_CORES):
        t1[c * NS:(c + 1) * NS] = (
            res.results[c]["t1hist"].reshape(S, T, NS).transpose(2, 1, 0)
        )

    return _backtrack(transitions, emissions, lengths, t1)


def _backtrack(transitions, emissions, lengths, t1):
    """Reference-exact backtrack from the t1 value history."""
    trans = transitions[:, :S, :]
    nb = np.arange(B)
    z = np.zeros((B, T), dtype=np.int32)
    last = lengths - 1
    z_last = np.argmax(t1[nb, last, :], axis=1).astype(np.int32)
    ptr = z_last.copy()
    for t in range(int(last.max()), 0, -1):
        at_last = (t == last)
        if at_last.any():
            ptr = np.where(at_last, z_last, ptr)
        z[:, t] = np.where(t <= last, ptr, 0)
        col = (t1[:, t - 1, :] + trans[nb, :, ptr]) + emissions[nb, t, ptr][:, None]
        ptr_new = np.argmax(col, axis=1).astype(np.int32)
        ptr = np.where(t <= last, ptr_new, ptr)
    z[:, 0] = ptr
    return z


# revision 4
# speedup vs baseline: 1.0469x; 1.0469x over previous
"""Batched Viterbi decode (CRF) on 8 Trainium2 NeuronCores — v3.

Same contract as the baseline (device computes bit-exact fp32 t1 history,
host backtracks).  Changes vs baseline:
  - adds: 2 ACT bias-adds + 1 DVE pair-add (free-broadcast AP) per 4-seq
    group, cutting the serial-ACT depth in each group's dependency cycle
    from 3 to 2 and total DVE instruction count
  - optional fp32r-mode PE transposes (1.5 cycles/row vs 2.0)
  - t1 history streamed to DRAM in chunks during the loop instead of one
    epilogue burst

Env knobs:
  V3_PAIR=1   use DVE pair-adds (else per-seq adds, ADD_ENG pattern)
  V3_F32R=1   bitcast transposes to float32r
  V3_ADD      fallback per-seq engine pattern when V3_PAIR=0
"""

import os
from contextlib import ExitStack

import numpy as np

S = 128
T = 2048
NS = 8
N_CORES = 8
B = NS * N_CORES

USE_PAIR = os.environ.get("V3_PAIR", "0") == "1"
USE_F32R = os.environ.get("V3_F32R", "0") == "1"
ADD_ENG = os.environ.get("V3_ADD", "aavaavaa")

_CACHE = {}


def _build_forward():
    import concourse.bacc as bacc
    import concourse.mybir as mybir
    import concourse.tile as tile

    F32 = mybir.dt.float32
    F32R = mybir.dt.float32r
    nc = bacc.Bacc("TRN2", num_devices=N_CORES)
    trans_in = nc.dram_tensor("transitions", [NS, S + 1, S], F32, kind="ExternalInput")
    em_in = nc.dram_tensor("emissions", [NS, T, S], F32, kind="ExternalInput")
    ident_in = nc.dram_tensor("identity", [S, S], F32, kind="ExternalInput")
    t1_out = nc.dram_tensor("t1hist", [S, T * NS], F32, kind="ExternalOutput")

    def tp(out_ap, in_ap, id_ap):
        if USE_F32R:
            nc.tensor.transpose(out_ap.bitcast(F32R), in_ap.bitcast(F32R),
                                id_ap.bitcast(F32R))
        else:
            nc.tensor.transpose(out_ap, in_ap, id_ap)

    with ExitStack() as ctx:
        trans_sb = ctx.enter_context(nc.sbuf_tensor([S, NS * S], F32))
        em_cols = ctx.enter_context(nc.sbuf_tensor([S, T * NS], F32))
        t1hist = ctx.enter_context(nc.sbuf_tensor([S, T * NS], F32))
        ident = ctx.enter_context(nc.sbuf_tensor([S, S], F32))
        start_sb = ctx.enter_context(nc.sbuf_tensor([S, NS], F32))
        em0_sb = ctx.enter_context(nc.sbuf_tensor([S, NS], F32))
        psum_pp = [
            [
                ctx.enter_context(nc.psum_tensor(f"pspp{k}g{g}", [S, NS * S // 2], F32))
                for g in range(2)
            ]
            for k in range(3)
        ]

        with tile.TileContext(nc) as tc, ExitStack() as pctx:
            sc_pool = pctx.enter_context(tc.tile_pool(name="scores", bufs=3))
            tmp_pool = pctx.enter_context(tc.tile_pool(name="tmp", bufs=3))
            stage_pool = pctx.enter_context(tc.tile_pool(name="stage", bufs=4))
            pst_pool = pctx.enter_context(tc.tile_pool(name="pst", bufs=2, space="PSUM"))

            # ---- prologue: transitions, identity, t1_0 ----
            for s in range(NS):
                nc.sync.dma_start(trans_sb[:, s * S:(s + 1) * S], trans_in[s, 0:S, :])
            nc.sync.dma_start(ident[:], ident_in[:])
            for s in range(NS):
                nc.sync.dma_start(
                    start_sb[:, s:s + 1], trans_in[s, S:S + 1, :].rearrange("o p -> p o")
                )
                nc.sync.dma_start(
                    em0_sb[:, s:s + 1], em_in[s, 0:1, :].rearrange("o p -> p o")
                )
            nc.vector.tensor_add(t1hist[:, 0:NS], start_sb[:], em0_sb[:])

            # ---- prologue: transpose emissions into em_cols[i, t*NS+s] ----
            for s in range(NS):
                for c in range(T // S):
                    stage = stage_pool.tile([S, S], F32, tag="emstage")
                    nc.sync.dma_start(stage[:], em_in[s, c * S:(c + 1) * S, :])
                    pst = pst_pool.tile([S, S], F32, tag="empsum")
                    nc.tensor.transpose(pst[:], stage[:], ident[:])
                    dst = em_cols[:, c * S * NS + s: (c + 1) * S * NS: NS]
                    nc.scalar.copy(dst, pst[:])

            # ---- main DP loop ----
            def step(t):
                base = (t - 1) * NS
                sc_tiles = {}
                for g in range(2):
                    s0 = g * 4
                    if USE_PAIR:
                        # 2 ACT singles + 1 DVE pair-add per group
                        for s in (s0, s0 + 1):
                            sc = sc_pool.tile([S, S], F32, tag=f"sc{s}")
                            nc.scalar.activation(
                                sc[:], trans_sb[:, s * S:(s + 1) * S],
                                mybir.ActivationFunctionType.Identity,
                                bias=t1hist[:, base + s:base + s + 1], scale=1.0,
                            )
                            sc_tiles[s] = sc
                        scp = sc_pool.tile([S, 2 * S], F32, tag=f"scp{g}")
                        tr2 = trans_sb[:, (s0 + 2) * S:(s0 + 4) * S].rearrange(
                            "p (s i) -> p s i", i=S)
                        t1p = t1hist[:, base + s0 + 2:base + s0 + 4, None].to_broadcast(
                            [S, 2, S])
                        nc.vector.tensor_add(
                            scp[:].rearrange("p (s i) -> p s i", i=S), tr2, t1p)
                        sc_tiles[s0 + 2] = None
                        sc_tiles[s0 + 3] = None
                        sc_tiles[(g, "pair")] = scp
                    else:
                        for s in range(s0, s0 + 4):
                            sc = sc_pool.tile([S, S], F32, tag=f"sc{s}")
                            t1col = t1hist[:, base + s:base + s + 1]
                            src = trans_sb[:, s * S:(s + 1) * S]
                            if ADD_ENG[s] == "a":
                                nc.scalar.activation(
                                    sc[:], src, mybir.ActivationFunctionType.Identity,
                                    bias=t1col, scale=1.0,
                                )
                            else:
                                nc.vector.tensor_scalar_add(sc[:], src, t1col)
                            sc_tiles[s] = sc

                tmp = tmp_pool.tile([S, NS], F32, tag="tmp")
                for g in range(2):
                    pst = psum_pp[t % 3][g]
                    for sl in range(4):
                        s = g * 4 + sl
                        if USE_PAIR and sl >= 2:
                            scp = sc_tiles[(g, "pair")]
                            src = scp[:, (sl - 2) * S:(sl - 1) * S]
                        else:
                            src = sc_tiles[s][:]
                        tp(pst[:, sl * S:(sl + 1) * S], src, ident[:])
                    pg = pst[:].rearrange("p (s i) -> p s i", i=S)
                    nc.vector.tensor_reduce(
                        tmp[:, g * 4:(g + 1) * 4], pg,
                        axis=mybir.AxisListType.X, op=mybir.AluOpType.max)
                    nc.vector.tensor_add(
                        t1hist[:, t * NS + g * 4:t * NS + (g + 1) * 4],
                        tmp[:, g * 4:(g + 1) * 4],
                        em_cols[:, t * NS + g * 4:t * NS + (g + 1) * 4])

            CHUNK = 256
            for t in range(1, T):
                step(t)
                if t % CHUNK == 0:
                    lo = (t - CHUNK) * NS
                    nc.sync.dma_start(
                        t1_out[:, lo:t * NS], t1hist[:, lo:t * NS])

            lo = (T // CHUNK * CHUNK - CHUNK) * NS
            nc.sync.dma_start(t1_out[:, lo:], t1hist[:, lo:])

    nc.finalize()
    return nc


def _get_nc():
    if "nc" not in _CACHE:
        _CACHE["nc"] = _build_forward()
    return _CACHE["nc"]


def kernel(transitions, emissions, lengths):
    from concourse.bass_utils import run_bass_kernel_spmd

    transitions = np.ascontiguousarray(transitions, dtype=np.float32)
    emissions = np.ascontiguousarray(emissions, dtype=np.float32)
    lengths = np.asarray(lengths, dtype=np.int32)
    assert transitions.shape == (B, S + 1, S)
    assert emissions.shape == (B, T, S)

    nc = _get_nc()
    eye = np.eye(S, dtype=np.float32)
    in_maps = [
        {
            "transitions": transitions[c * NS:(c + 1) * NS],
            "emissions": emissions[c * NS:(c + 1) * NS],
            "identity": eye,
        }
        for c in range(N_CORES)
    ]
    res = run_bass_kernel_spmd(
        nc, in_maps, core_ids=list(range(N_CORES)),
        trace=bool(os.environ.get("VIT_TRACE")),
    )
    if os.environ.get("VIT_TRACE"):
        _CACHE["last_exec_time_ns"] = res.exec_time_ns
        _CACHE["last_res"] = res

    t1 = np.empty((B, T, S), dtype=np.float32)
    for c in range(N_CORES):
        t1[c * NS:(c + 1) * NS] = (
            res.results[c]["t1hist"].reshape(S, T, NS).transpose(2, 1, 0)
        )

    return _backtrack(transitions, emissions, lengths, t1)


def _backtrack(transitions, emissions, lengths, t1):
    """Reference-exact backtrack from the t1 value history."""
    trans = transitions[:, :S, :]
    nb = np.arange(B)
    z = np.zeros((B, T), dtype=np.int32)
    last = lengths - 1
    z_last = np.argmax(t1[nb, last, :], axis=1).astype(np.int32)
    ptr = z_last.copy()
    for t in range(int(last.max()), 0, -1):
        at_last = (t == last)
        if at_last.any():
            ptr = np.where(at_last, z_last, ptr)
        z[:, t] = np.where(t <= last, ptr, 0)
        col = (t1[:, t - 1, :] + trans[nb, :, ptr]) + emissions[nb, t, ptr][:, None]
        ptr_new = np.argmax(col, axis=1).astype(np.int32)
        ptr = np.where(t <= last, ptr_new, ptr)
    z[:, 0] = ptr
    return z
